# revision 6
# baseline (speedup 1.0000x reference)
"""BCQ linear kernel for 8 TRN2 NeuronCores.

y = x @ dequant(qweight, alpha, beta)
  x: (4, 2048, 4096) f32, qweight: (128, 4, 4096) i32 bit-planes,
  alpha: (32, 4, 4096) f32, beta: (32, 4096) f32 -> y: (4, 2048, 4096) f32

Strategy: tensor-parallel over out_features (512 per core). Host folds the
BCQ scales into two pre-paired sign planes per weight element:
    v2[k,0,o] = alpha[g,0,o]*s0 + alpha[g,1,o]*s1 + beta[g,o]/2
    v2[k,1,o] = alpha[g,2,o]*s2 + alpha[g,3,o]*s3 + beta[g,o]/2
so the on-chip dequant is a single bf16 add per 128-row k-tile:
    w[k,o] = v2[k,0,o] + v2[k,1,o]
(8.4 MB of v2 DMA per core vs 33.5 MB of int32 sign words + 20 MB of
alpha broadcast in the previous scheme -- phase 1 is no longer DMA-bound
and the PE stays out of the mid p-state.)

Schedule per core:
  - phase 1: while the 32 v2 k-tiles stream in (gpsimd DGE queue) and fold
    into resident bf16 w tiles, the first 2 m-chunks (8 m-tiles = 8 PSUM
    banks) run k-outer against the dequant frontier. x for these chunks
    arrives as 8 quarter-tiles so the first matmul can issue at ~4us.
  - phase 2: remaining 14 chunks sweep k-inner at the steady 216ns/MM
    pitch (moving=512, LDWEIGHTS pull-ahead).
  - out tiles are cast to bf16 on the scalar engine (PSUM->SBUF) and
    DMA'd out on the scalar DGE queue; host casts back to f32.
Host gathers the 8 out-feature slices.
"""
import sys

if "/opt/trn_rl_repo" not in sys.path:
    sys.path.insert(0, "/opt/trn_rl_repo")

import numpy as np
from ml_dtypes import bfloat16

import concourse.bacc as bacc
import concourse.tile as tile
from concourse import mybir
from concourse.bass_utils import run_bass_kernel_spmd

IN_F = 4096
OUT_F = 4096
GROUP_SIZE = 128
WB = 4
BATCH = 4
SEQ = 2048
M_FULL = BATCH * SEQ          # 8192
N_CORES = 8
O_SH = OUT_F // N_CORES       # 512
P = 128

F32 = mybir.dt.float32
BF16 = mybir.dt.bfloat16
Alu = mybir.AluOpType


def build(M=M_FULL, K=IN_F, O=O_SH, debug=False):
    """Build the per-core Bass graph (SPMD: same graph, per-core inputs)."""
    assert M % 512 == 0 and K % P == 0
    KT = K // P                # k tiles (= quant groups, GROUP_SIZE == P)
    MC = M // 512              # m chunks of 512 rows (4 m-tiles each)
    P1C = min(2, MC)           # chunks processed k-outer during dequant
    QK = 4                     # k-tiles per phase-1 x slice tile
    PREF = min(12, KT)         # v2 tiles prefetched ahead of the fold

    nc = bacc.Bacc(None, target_bir_lowering=False, debug=debug)

    xt_d = nc.dram_tensor("xt", (MC, P, KT, 512), BF16, kind="ExternalInput")
    v2_d = nc.dram_tensor("v2", (KT, P, 2, O), BF16, kind="ExternalInput")
    out_d = nc.dram_tensor("out", (M, O), BF16, kind="ExternalOutput")

    with tile.TileContext(nc) as tc:
        with (
            tc.tile_pool(name="wpool", bufs=1) as wpool,
            tc.tile_pool(name="vin", bufs=PREF) as vin,
            tc.tile_pool(name="xq", bufs=1) as xq,
            tc.tile_pool(name="xs", bufs=2) as xs,
            tc.tile_pool(name="ys", bufs=4) as ys,
            tc.tile_pool(name="ps", bufs=8, space="PSUM") as ps,
        ):
            w_tiles = [
                wpool.tile([P, O], BF16, name=f"w{g}", tag=f"w{g}")
                for g in range(KT)
            ]

            vts = {}

            def load_v2(g):
                # scalar-engine HWDGE queue: starts early (gpsimd SWDGE
                # costs ~1us/descriptor and delays the first fold by ~7us)
                vt = vin.tile([P, 2, O], BF16, name=f"v{g}", tag="v")
                nc.scalar.dma_start(out=vt[:], in_=v2_d[g])
                vts[g] = vt

            for g in range(PREF):
                load_v2(g)

            # phase-1 x: small k-slices, interleaved across the P1C chunks
            # in k-order so the first matmuls can start at ~7us
            x_q = {}
            for q in range(KT // QK):
                for mc in range(P1C):
                    xt_sb = xq.tile([P, QK, 512], BF16, name=f"xq{mc}_{q}",
                                    tag=f"xq{mc}_{q}")
                    nc.sync.dma_start(
                        out=xt_sb[:], in_=xt_d[mc, :, q * QK:(q + 1) * QK, :]
                    )
                    x_q[(mc, q)] = xt_sb

            psum_p1 = [
                ps.tile([P, O], F32, name=f"ps{i}", tag="ps")
                for i in range(4 * P1C)
            ]

            # ---- phase 1: fold v2 k-tiles; matmul first P1C chunks k-outer ----
            for g in range(KT):
                vt = vts[g]
                nc.vector.tensor_tensor(
                    w_tiles[g][:], vt[:, 0, :], vt[:, 1, :], Alu.add
                )
                if g + PREF < KT:
                    load_v2(g + PREF)

                for mc in range(P1C):
                    xt_sb = x_q[(mc, g // QK)]
                    for mt in range(4):
                        nc.tensor.matmul(
                            psum_p1[mc * 4 + mt][:],
                            xt_sb[:, g % QK, mt * 128:(mt + 1) * 128],
                            w_tiles[g][:],
                            start=(g == 0),
                            stop=(g == KT - 1),
                        )

            for mc in range(P1C):
                for mt in range(4):
                    y_sb = ys.tile([P, O], BF16, tag="y")
                    nc.scalar.copy(y_sb[:], psum_p1[mc * 4 + mt][:])
                    row = (mc * 4 + mt) * 128
                    nc.scalar.dma_start(out=out_d[row:row + 128, :], in_=y_sb[:])

            # ---- phase 2: remaining m chunks at full speed ----
            for mc in range(P1C, MC):
                xt_sb = xs.tile([P, KT, 512], BF16, name=f"xt_sb{mc}", tag="xt")
                nc.sync.dma_start(out=xt_sb[:], in_=xt_d[mc])
                for mt in range(4):
                    psum = ps.tile([P, O], F32, tag="ps")
                    for g in range(KT):
                        nc.tensor.matmul(
                            psum[:],
                            xt_sb[:, g, mt * 128:(mt + 1) * 128],
                            w_tiles[g][:],
                            start=(g == 0),
                            stop=(g == KT - 1),
                        )
                    y_sb = ys.tile([P, O], BF16, tag="y")
                    nc.scalar.copy(y_sb[:], psum[:])
                    row = (mc * 4 + mt) * 128
                    nc.scalar.dma_start(out=out_d[row:row + 128, :], in_=y_sb[:])

    return nc


def host_prep(x, qweight, alpha, beta, M=M_FULL, K=IN_F):
    """Full inputs -> per-core in_maps (shard over out_features)."""
    KT = K // P
    MC = M // 512
    x3 = x.reshape(M, K).astype(bfloat16)
    # (MC, P, KT, 512): per-partition-contiguous chunk tiles for fast DMA
    x2 = np.ascontiguousarray(
        x3.reshape(MC, 512, KT, P).transpose(0, 3, 2, 1)
    )

    k = np.arange(K)
    widx = (k // 32).astype(np.int64)
    shr = (k % 32).astype(np.int32)
    gidx = (k // GROUP_SIZE).astype(np.int64)

    o_sh = qweight.shape[-1] // N_CORES
    in_maps = []
    for c in range(N_CORES):
        sl = slice(c * o_sh, (c + 1) * o_sh)
        qw_s = qweight[:, :, sl]                       # (K/32, WB, o_sh) i32
        signs = (
            ((qw_s[widx] >> shr[:, None, None]) & 1).astype(np.float32) * 2.0
            - 1.0
        )                                              # (K, WB, o_sh) {-1,+1}
        al_s = alpha[:, :, sl].astype(np.float32)[gidx]   # (K, WB, o_sh)
        hb = 0.5 * beta[:, sl].astype(np.float32)[gidx]   # (K, o_sh)
        va = signs * al_s
        v2 = np.empty((K, 2, o_sh), dtype=np.float32)
        v2[:, 0, :] = va[:, 0, :] + va[:, 1, :] + hb
        v2[:, 1, :] = va[:, 2, :] + va[:, 3, :] + hb
        v2 = np.ascontiguousarray(
            v2.reshape(KT, P, 2, o_sh).astype(bfloat16)
        )
        in_maps.append({"xt": x2, "v2": v2})
    return in_maps


_NC_CACHE = {}


def _get_nc():
    if "nc" not in _NC_CACHE:
        nc = build()
        nc.compile()
        _NC_CACHE["nc"] = nc
    return _NC_CACHE["nc"]


def run(x, qweight, alpha, beta, trace=False, **kwargs):
    nc = _get_nc()
    in_maps = host_prep(x, qweight, alpha, beta)
    res = run_bass_kernel_spmd(
        nc, in_maps, core_ids=list(range(N_CORES)), trace=trace, **kwargs
    )
    y = np.concatenate(
        [np.asarray(res.results[c]["out"]) for c in range(N_CORES)], axis=1
    )
    y = np.ascontiguousarray(y.astype(np.float32)).reshape(BATCH, SEQ, OUT_F)
    return y, res


def kernel(x, qweight, alpha, beta):
    y, _ = run(
        np.asarray(x), np.asarray(qweight), np.asarray(alpha), np.asarray(beta)
    )
    return y


# revision 8
# speedup vs baseline: 1.1878x; 1.1878x over previous
"""BCQ linear kernel for 8 TRN2 NeuronCores.

y = x @ dequant(qweight, alpha, beta)
  x: (4, 2048, 4096) f32, qweight: (128, 4, 4096) i32 bit-planes,
  alpha: (32, 4, 4096) f32, beta: (32, 4096) f32 -> y: (4, 2048, 4096) f32

Strategy: tensor-parallel over out_features (512 per core). Host folds the
BCQ scales into two pre-paired sign planes per weight element:
    v2[k,0,o] = alpha[g,0,o]*s0 + alpha[g,1,o]*s1 + beta[g,o]/2
    v2[k,1,o] = alpha[g,2,o]*s2 + alpha[g,3,o]*s3 + beta[g,o]/2
so the on-chip dequant is a single bf16 add per 128-row k-tile:
    w[k,o] = v2[k,0,o] + v2[k,1,o]
(8.4 MB of v2 DMA per core vs 33.5 MB of int32 sign words + 20 MB of
alpha broadcast in the previous scheme -- phase 1 is no longer DMA-bound
and the PE stays out of the mid p-state.)

Schedule per core:
  - phase 1: while the 32 v2 k-tiles stream in (gpsimd DGE queue) and fold
    into resident bf16 w tiles, the first 2 m-chunks (8 m-tiles = 8 PSUM
    banks) run k-outer against the dequant frontier. x for these chunks
    arrives as 8 quarter-tiles so the first matmul can issue at ~4us.
  - phase 2: remaining 14 chunks sweep k-inner at the steady 216ns/MM
    pitch (moving=512, LDWEIGHTS pull-ahead).
  - out tiles are cast to bf16 on the scalar engine (PSUM->SBUF) and
    DMA'd out on the scalar DGE queue; host casts back to f32.
Host gathers the 8 out-feature slices.
"""
import sys

if "/opt/trn_rl_repo" not in sys.path:
    sys.path.insert(0, "/opt/trn_rl_repo")

import numpy as np
from ml_dtypes import bfloat16

import concourse.bacc as bacc
import concourse.tile as tile
from concourse import mybir
from concourse.bass_utils import run_bass_kernel_spmd

IN_F = 4096
OUT_F = 4096
GROUP_SIZE = 128
WB = 4
BATCH = 4
SEQ = 2048
M_FULL = BATCH * SEQ          # 8192
N_CORES = 8
O_SH = OUT_F // N_CORES       # 512
P = 128

F32 = mybir.dt.float32
BF16 = mybir.dt.bfloat16
Alu = mybir.AluOpType


def build(M=M_FULL, K=IN_F, O=O_SH, debug=False):
    """Build the per-core Bass graph (SPMD: same graph, per-core inputs)."""
    assert M % 512 == 0 and K % P == 0
    KT = K // P                # k tiles (= quant groups, GROUP_SIZE == P)
    MC = M // 512              # m chunks of 512 rows (4 m-tiles each)
    P1C = min(2, MC)           # chunks processed k-outer during dequant
    QK = 4                     # k-tiles per phase-1 x slice tile
    PREF = min(12, KT)         # v2 tiles prefetched ahead of the fold

    nc = bacc.Bacc(None, target_bir_lowering=False, debug=debug)

    xt_d = nc.dram_tensor("xt", (MC, P, KT, 512), BF16, kind="ExternalInput")
    v2_d = nc.dram_tensor("v2", (KT, P, 2, O), BF16, kind="ExternalInput")
    out_d = nc.dram_tensor("out", (M, O), BF16, kind="ExternalOutput")

    with tile.TileContext(nc) as tc:
        with (
            tc.tile_pool(name="wpool", bufs=1) as wpool,
            tc.tile_pool(name="vin", bufs=PREF) as vin,
            tc.tile_pool(name="xq", bufs=1) as xq,
            tc.tile_pool(name="xs", bufs=2) as xs,
            tc.tile_pool(name="ys", bufs=4) as ys,
            tc.tile_pool(name="ps", bufs=8, space="PSUM") as ps,
        ):
            w_tiles = [
                wpool.tile([P, O], BF16, name=f"w{g}", tag=f"w{g}")
                for g in range(KT)
            ]

            vts = {}

            def load_v2(g, eng=None):
                # scalar-engine HWDGE queue: starts early (gpsimd SWDGE
                # costs ~1us/descriptor and delays the first fold by ~7us);
                # the very first tile rides the sync queue ahead of the x
                # slices so the fold chain starts as soon as DMA is up
                vt = vin.tile([P, 2, O], BF16, name=f"v{g}", tag="v")
                (eng or nc.scalar).dma_start(out=vt[:], in_=v2_d[g])
                vts[g] = vt

            load_v2(0, eng=nc.sync)
            for g in range(1, PREF):
                load_v2(g)

            # phase-1 x: small k-slices, interleaved across the P1C chunks
            # in k-order so the first matmuls can start at ~7us
            x_q = {}
            for q in range(KT // QK):
                for mc in range(P1C):
                    xt_sb = xq.tile([P, QK, 512], BF16, name=f"xq{mc}_{q}",
                                    tag=f"xq{mc}_{q}")
                    nc.sync.dma_start(
                        out=xt_sb[:], in_=xt_d[mc, :, q * QK:(q + 1) * QK, :]
                    )
                    x_q[(mc, q)] = xt_sb

            psum_p1 = [
                ps.tile([P, O], F32, name=f"ps{i}", tag="ps")
                for i in range(4 * P1C)
            ]

            # ---- phase 1: fold v2 k-tiles; matmul first P1C chunks k-outer ----
            for g in range(KT):
                vt = vts[g]
                nc.vector.tensor_tensor(
                    w_tiles[g][:], vt[:, 0, :], vt[:, 1, :], Alu.add
                )
                if g + PREF < KT:
                    load_v2(g + PREF)

                for mc in range(P1C):
                    xt_sb = x_q[(mc, g // QK)]
                    for mt in range(4):
                        nc.tensor.matmul(
                            psum_p1[mc * 4 + mt][:],
                            xt_sb[:, g % QK, mt * 128:(mt + 1) * 128],
                            w_tiles[g][:],
                            start=(g == 0),
                            stop=(g == KT - 1),
                        )

            for mc in range(P1C):
                for mt in range(4):
                    y_sb = ys.tile([P, O], BF16, tag="y")
                    nc.scalar.copy(y_sb[:], psum_p1[mc * 4 + mt][:])
                    row = (mc * 4 + mt) * 128
                    nc.scalar.dma_start(out=out_d[row:row + 128, :], in_=y_sb[:])

            # ---- phase 2: remaining m chunks at full speed ----
            for mc in range(P1C, MC):
                xt_sb = xs.tile([P, KT, 512], BF16, name=f"xt_sb{mc}", tag="xt")
                nc.sync.dma_start(out=xt_sb[:], in_=xt_d[mc])
                for mt in range(4):
                    psum = ps.tile([P, O], F32, tag="ps")
                    for g in range(KT):
                        nc.tensor.matmul(
                            psum[:],
                            xt_sb[:, g, mt * 128:(mt + 1) * 128],
                            w_tiles[g][:],
                            start=(g == 0),
                            stop=(g == KT - 1),
                        )
                    y_sb = ys.tile([P, O], BF16, tag="y")
                    nc.scalar.copy(y_sb[:], psum[:])
                    row = (mc * 4 + mt) * 128
                    nc.scalar.dma_start(out=out_d[row:row + 128, :], in_=y_sb[:])

    return nc


def host_prep(x, qweight, alpha, beta, M=M_FULL, K=IN_F):
    """Full inputs -> per-core in_maps (shard over out_features)."""
    KT = K // P
    MC = M // 512
    x3 = x.reshape(M, K).astype(bfloat16)
    # (MC, P, KT, 512): per-partition-contiguous chunk tiles for fast DMA
    x2 = np.ascontiguousarray(
        x3.reshape(MC, 512, KT, P).transpose(0, 3, 2, 1)
    )

    k = np.arange(K)
    widx = (k // 32).astype(np.int64)
    shr = (k % 32).astype(np.int32)
    gidx = (k // GROUP_SIZE).astype(np.int64)

    o_sh = qweight.shape[-1] // N_CORES
    in_maps = []
    for c in range(N_CORES):
        sl = slice(c * o_sh, (c + 1) * o_sh)
        qw_s = qweight[:, :, sl]                       # (K/32, WB, o_sh) i32
        signs = (
            ((qw_s[widx] >> shr[:, None, None]) & 1).astype(np.float32) * 2.0
            - 1.0
        )                                              # (K, WB, o_sh) {-1,+1}
        al_s = alpha[:, :, sl].astype(np.float32)[gidx]   # (K, WB, o_sh)
        hb = 0.5 * beta[:, sl].astype(np.float32)[gidx]   # (K, o_sh)
        va = signs * al_s
        v2 = np.empty((K, 2, o_sh), dtype=np.float32)
        v2[:, 0, :] = va[:, 0, :] + va[:, 1, :] + hb
        v2[:, 1, :] = va[:, 2, :] + va[:, 3, :] + hb
        v2 = np.ascontiguousarray(
            v2.reshape(KT, P, 2, o_sh).astype(bfloat16)
        )
        in_maps.append({"xt": x2, "v2": v2})
    return in_maps


_NC_CACHE = {}


def _get_nc():
    if "nc" not in _NC_CACHE:
        nc = build()
        nc.compile()
        _NC_CACHE["nc"] = nc
    return _NC_CACHE["nc"]


def run(x, qweight, alpha, beta, trace=False, **kwargs):
    nc = _get_nc()
    in_maps = host_prep(x, qweight, alpha, beta)
    res = run_bass_kernel_spmd(
        nc, in_maps, core_ids=list(range(N_CORES)), trace=trace, **kwargs
    )
    y = np.concatenate(
        [np.asarray(res.results[c]["out"]) for c in range(N_CORES)], axis=1
    )
    y = np.ascontiguousarray(y.astype(np.float32)).reshape(BATCH, SEQ, OUT_F)
    return y, res


def kernel(x, qweight, alpha, beta):
    y, _ = run(
        np.asarray(x), np.asarray(qweight), np.asarray(alpha), np.asarray(beta)
    )
    return y


# revision 9
# speedup vs baseline: 1.1912x; 1.0029x over previous
"""BCQ linear kernel for 8 TRN2 NeuronCores.

y = x @ dequant(qweight, alpha, beta)
  x: (4, 2048, 4096) f32, qweight: (128, 4, 4096) i32 bit-planes,
  alpha: (32, 4, 4096) f32, beta: (32, 4096) f32 -> y: (4, 2048, 4096) f32

Strategy: tensor-parallel over out_features (512 per core). Host folds the
BCQ scales into two pre-paired sign planes per weight element:
    v2[k,0,o] = alpha[g,0,o]*s0 + alpha[g,1,o]*s1 + beta[g,o]/2
    v2[k,1,o] = alpha[g,2,o]*s2 + alpha[g,3,o]*s3 + beta[g,o]/2
so the on-chip dequant is a single bf16 add per 128-row k-tile:
    w[k,o] = v2[k,0,o] + v2[k,1,o]
(8.4 MB of v2 DMA per core vs 33.5 MB of int32 sign words + 20 MB of
alpha broadcast in the previous scheme -- phase 1 is no longer DMA-bound
and the PE stays out of the mid p-state.)

Schedule per core:
  - phase 1: while the 32 v2 k-tiles stream in (gpsimd DGE queue) and fold
    into resident bf16 w tiles, the first 2 m-chunks (8 m-tiles = 8 PSUM
    banks) run k-outer against the dequant frontier. x for these chunks
    arrives as 8 quarter-tiles so the first matmul can issue at ~4us.
  - phase 2: remaining 14 chunks sweep k-inner at the steady 216ns/MM
    pitch (moving=512, LDWEIGHTS pull-ahead).
  - out tiles are cast to bf16 on the scalar engine (PSUM->SBUF) and
    DMA'd out on the scalar DGE queue; host casts back to f32.
Host gathers the 8 out-feature slices.
"""
import sys

if "/opt/trn_rl_repo" not in sys.path:
    sys.path.insert(0, "/opt/trn_rl_repo")

import numpy as np
from ml_dtypes import bfloat16

import concourse.bacc as bacc
import concourse.tile as tile
from concourse import mybir
from concourse.bass_utils import run_bass_kernel_spmd

IN_F = 4096
OUT_F = 4096
GROUP_SIZE = 128
WB = 4
BATCH = 4
SEQ = 2048
M_FULL = BATCH * SEQ          # 8192
N_CORES = 8
O_SH = OUT_F // N_CORES       # 512
P = 128

F32 = mybir.dt.float32
BF16 = mybir.dt.bfloat16
Alu = mybir.AluOpType


def build(M=M_FULL, K=IN_F, O=O_SH, debug=False):
    """Build the per-core Bass graph (SPMD: same graph, per-core inputs)."""
    assert M % 512 == 0 and K % P == 0
    KT = K // P                # k tiles (= quant groups, GROUP_SIZE == P)
    MC = M // 512              # m chunks of 512 rows (4 m-tiles each)
    P1C = min(2, MC)           # chunks processed k-outer during dequant
    QK = 4                     # k-tiles per phase-1 x slice tile
    PREF = min(12, KT)         # v2 tiles prefetched ahead of the fold

    nc = bacc.Bacc(None, target_bir_lowering=False, debug=debug)

    xt_d = nc.dram_tensor("xt", (MC, P, KT, 512), BF16, kind="ExternalInput")
    v2_d = nc.dram_tensor("v2", (KT, P, 2, O), BF16, kind="ExternalInput")
    out_d = nc.dram_tensor("out", (M, O), BF16, kind="ExternalOutput")

    with tile.TileContext(nc) as tc:
        with (
            tc.tile_pool(name="wpool", bufs=1) as wpool,
            tc.tile_pool(name="vin", bufs=PREF) as vin,
            tc.tile_pool(name="xq", bufs=1) as xq,
            tc.tile_pool(name="xs", bufs=2) as xs,
            tc.tile_pool(name="ys", bufs=4) as ys,
            tc.tile_pool(name="ps", bufs=8, space="PSUM") as ps,
        ):
            w_tiles = [
                wpool.tile([P, O], BF16, name=f"w{g}", tag=f"w{g}")
                for g in range(KT)
            ]

            vts = {}

            def load_v2(g, eng=None):
                # scalar-engine HWDGE queue: starts early (gpsimd SWDGE
                # costs ~1us/descriptor and delays the first fold by ~7us);
                # the very first tile rides the sync queue ahead of the x
                # slices so the fold chain starts as soon as DMA is up
                vt = vin.tile([P, 2, O], BF16, name=f"v{g}", tag="v")
                (eng or nc.scalar).dma_start(out=vt[:], in_=v2_d[g])
                vts[g] = vt

            for g in range(3):
                load_v2(g, eng=nc.sync)
            for g in range(3, PREF):
                load_v2(g)

            # phase-1 x: small k-slices, interleaved across the P1C chunks
            # in k-order so the first matmuls can start at ~7us
            x_q = {}
            for q in range(KT // QK):
                for mc in range(P1C):
                    xt_sb = xq.tile([P, QK, 512], BF16, name=f"xq{mc}_{q}",
                                    tag=f"xq{mc}_{q}")
                    nc.sync.dma_start(
                        out=xt_sb[:], in_=xt_d[mc, :, q * QK:(q + 1) * QK, :]
                    )
                    x_q[(mc, q)] = xt_sb

            psum_p1 = [
                ps.tile([P, O], F32, name=f"ps{i}", tag="ps")
                for i in range(4 * P1C)
            ]

            # ---- phase 1: fold v2 k-tiles; matmul first P1C chunks k-outer ----
            for g in range(KT):
                vt = vts[g]
                nc.vector.tensor_tensor(
                    w_tiles[g][:], vt[:, 0, :], vt[:, 1, :], Alu.add
                )
                if g + PREF < KT:
                    load_v2(g + PREF)

                for mc in range(P1C):
                    xt_sb = x_q[(mc, g // QK)]
                    for mt in range(4):
                        nc.tensor.matmul(
                            psum_p1[mc * 4 + mt][:],
                            xt_sb[:, g % QK, mt * 128:(mt + 1) * 128],
                            w_tiles[g][:],
                            start=(g == 0),
                            stop=(g == KT - 1),
                        )

            for mc in range(P1C):
                for mt in range(4):
                    y_sb = ys.tile([P, O], BF16, tag="y")
                    nc.scalar.copy(y_sb[:], psum_p1[mc * 4 + mt][:])
                    row = (mc * 4 + mt) * 128
                    nc.scalar.dma_start(out=out_d[row:row + 128, :], in_=y_sb[:])

            # ---- phase 2: remaining m chunks at full speed ----
            for mc in range(P1C, MC):
                xt_sb = xs.tile([P, KT, 512], BF16, name=f"xt_sb{mc}", tag="xt")
                nc.sync.dma_start(out=xt_sb[:], in_=xt_d[mc])
                for mt in range(4):
                    psum = ps.tile([P, O], F32, tag="ps")
                    for g in range(KT):
                        nc.tensor.matmul(
                            psum[:],
                            xt_sb[:, g, mt * 128:(mt + 1) * 128],
                            w_tiles[g][:],
                            start=(g == 0),
                            stop=(g == KT - 1),
                        )
                    y_sb = ys.tile([P, O], BF16, tag="y")
                    nc.scalar.copy(y_sb[:], psum[:])
                    row = (mc * 4 + mt) * 128
                    nc.scalar.dma_start(out=out_d[row:row + 128, :], in_=y_sb[:])

    return nc


def host_prep(x, qweight, alpha, beta, M=M_FULL, K=IN_F):
    """Full inputs -> per-core in_maps (shard over out_features)."""
    KT = K // P
    MC = M // 512
    x3 = x.reshape(M, K).astype(bfloat16)
    # (MC, P, KT, 512): per-partition-contiguous chunk tiles for fast DMA
    x2 = np.ascontiguousarray(
        x3.reshape(MC, 512, KT, P).transpose(0, 3, 2, 1)
    )

    k = np.arange(K)
    widx = (k // 32).astype(np.int64)
    shr = (k % 32).astype(np.int32)
    gidx = (k // GROUP_SIZE).astype(np.int64)

    o_sh = qweight.shape[-1] // N_CORES
    in_maps = []
    for c in range(N_CORES):
        sl = slice(c * o_sh, (c + 1) * o_sh)
        qw_s = qweight[:, :, sl]                       # (K/32, WB, o_sh) i32
        signs = (
            ((qw_s[widx] >> shr[:, None, None]) & 1).astype(np.float32) * 2.0
            - 1.0
        )                                              # (K, WB, o_sh) {-1,+1}
        al_s = alpha[:, :, sl].astype(np.float32)[gidx]   # (K, WB, o_sh)
        hb = 0.5 * beta[:, sl].astype(np.float32)[gidx]   # (K, o_sh)
        va = signs * al_s
        v2 = np.empty((K, 2, o_sh), dtype=np.float32)
        v2[:, 0, :] = va[:, 0, :] + va[:, 1, :] + hb
        v2[:, 1, :] = va[:, 2, :] + va[:, 3, :] + hb
        v2 = np.ascontiguousarray(
            v2.reshape(KT, P, 2, o_sh).astype(bfloat16)
        )
        in_maps.append({"xt": x2, "v2": v2})
    return in_maps


_NC_CACHE = {}


def _get_nc():
    if "nc" not in _NC_CACHE:
        nc = build()
        nc.compile()
        _NC_CACHE["nc"] = nc
    return _NC_CACHE["nc"]


def run(x, qweight, alpha, beta, trace=False, **kwargs):
    nc = _get_nc()
    in_maps = host_prep(x, qweight, alpha, beta)
    res = run_bass_kernel_spmd(
        nc, in_maps, core_ids=list(range(N_CORES)), trace=trace, **kwargs
    )
    y = np.concatenate(
        [np.asarray(res.results[c]["out"]) for c in range(N_CORES)], axis=1
    )
    y = np.ascontiguousarray(y.astype(np.float32)).reshape(BATCH, SEQ, OUT_F)
    return y, res


def kernel(x, qweight, alpha, beta):
    y, _ = run(
        np.asarray(x), np.asarray(qweight), np.asarray(alpha), np.asarray(beta)
    )
    return y
